# revision 2
# baseline (speedup 1.0000x reference)
"""Trainium2 Bass kernel v2 for nn_AggregationAndDiscriminationLoss.

Data-parallel over batch: 2 images per core on 8 cores.

v2 changes vs baseline:
  - PE runs fp8e4 DoubleRow matmuls: the kt axis pairs pixel columns
    (c, c+392) within each 784-col chunk, so each output column contracts
    256 pixels instead of 128 (2.1x measured PE speedup).
  - Mask tiles stay bf16 (DVE is_equal runs in 4x perf mode); the PE reads
    them through an fp8 bitcast view of the odd bytes: bf16 1.0 = 0x3F80,
    whose high byte 0x3F is fp8e4 1.875.  All PSUM sums are uniformly
    scaled by 1.875, divided out on host.
  - Stationary values (v = sum_c sim_c^2 and v*(T==K)) are fp8e4, written
    by ACT copy passes in the DoubleRow pair layout.

Decode identical to the baseline (M=113 = v(56) | ones | vtk(56),
PSUM [113, 2, 448] per image per set) plus the 1/1.875 scale.
"""

import numpy as np

import concourse.bass as bass
import concourse.tile as tile
from concourse import mybir
from concourse.bass_utils import run_bass_kernel_spmd

B, C, H, W = 16, 4, 896, 896
NCORES = 8
IMGS = B // NCORES          # images per core
P = 128
NFREE = (H * W) // P        # 6272
NCH = 8                     # chunks per image
CF = NFREE // NCH           # 784 columns per chunk
HF = CF // 2                # 392: kt pairs are (c, c+HF)
BC = 56                     # stationary value-pair block width
NB = HF // BC               # 7 blocks per chunk
M = 2 * BC + 1              # 113 stationary columns: v | ones | vtk
K_MAX = 16
SIGMA_AGG = 0.5
SIGMA_DIS = 3.0
MSCALE = 1.875              # fp8e4 value of bf16(1.0)'s high byte


def _legalize_sync(nc):
    """Split >1-wait instructions: this walrus only encodes one sync wait."""
    for fn in nc.m.functions:
        for blk in fn.blocks:
            new = []
            for ins in blk.instructions:
                si = ins.sync_info
                if si is not None and len(si.on_wait) > 1:
                    waits = list(si.on_wait)
                    for k, w in enumerate(waits[:-1]):
                        nop = mybir.InstNoOp(name=f"{ins.name}-ws{k}", ins=[], outs=[])
                        nop.engine = ins.engine
                        nop.sync_info = mybir.SyncInfo(on_wait=[w], on_update=[])
                        new.append(nop)
                    ins.sync_info = mybir.SyncInfo(
                        on_wait=[waits[-1]], on_update=list(si.on_update)
                    )
                new.append(ins)
            blk.instructions = new


def _build_nc(reps=1, ablate=(), nlab=16, pe_sets=2, sq_on=True, cv_on=True, tk_on=True):
    nc = bass.Bass()
    dt = mybir.dt
    eq = mybir.AluOpType.is_equal
    DR = mybir.MatmulPerfMode.DoubleRow

    sim = nc.dram_tensor("sim", [IMGS, C, P, NFREE], dt.float32, kind="ExternalInput")
    tl = nc.dram_tensor("tlab", [IMGS, P, NFREE], dt.int32, kind="ExternalInput")
    kl = nc.dram_tensor("klab", [IMGS, P, NFREE], dt.int32, kind="ExternalInput")
    acc_d = nc.dram_tensor("acc", [IMGS, M, 2, 448], dt.float32, kind="ExternalOutput")
    accK_d = nc.dram_tensor("accK", [IMGS, M, 2, 448], dt.float32, kind="ExternalOutput")

    with tile.TileContext(nc) as tc:
        with (
            tc.tile_pool(name="io", bufs=3) as io,
            tc.tile_pool(name="tmp", bufs=2) as tmp,
            tc.tile_pool(name="mks", bufs=2) as mks,
            tc.tile_pool(name="tkp", bufs=3) as tkp,
            tc.tile_pool(name="vtp", bufs=2) as vtp,
            tc.tile_pool(name="ps", bufs=2, space="PSUM") as ps,
        ):
            def _image(b):
                poT = ps.tile([M, 2, 512], dt.float32, tag="poT")
                poK = ps.tile([M, 2, 512], dt.float32, tag="poK")
                state = {}

                def phase_load(ci):
                    cs = slice(ci * CF, (ci + 1) * CF)
                    ti = io.tile([P, CF], dt.int32, tag="ti")
                    nc.sync.dma_start(ti[:], tl[b, :, cs])
                    ki = io.tile([P, CF], dt.int32, tag="ki")
                    nc.sync.dma_start(ki[:], kl[b, :, cs])
                    ch4 = io.tile([P, C, CF], dt.float32, tag="ch4")
                    nc.sync.dma_start(
                        ch4[:], sim[b, :, :, cs].rearrange("c p f -> p c f")
                    )
                    chs = [ch4[:, c, :] for c in range(C)]
                    tkb = tkp.tile([P, 2, CF], dt.bfloat16, tag="tkb")
                    if tk_on:
                        nc.scalar.copy(tkb[:, 0, :], ti[:])
                        nc.scalar.copy(tkb[:, 1, :], ki[:])
                    else:
                        nc.vector.memset(tkb[:, 0, 0:1], 1.0)
                    sqs = []
                    for c in range(C):
                        sq = tmp.tile([P, CF], dt.bfloat16, tag=f"sq{c}")
                        if sq_on:
                            nc.scalar.activation(
                                sq[:], chs[c], mybir.ActivationFunctionType.Square
                            )
                        else:
                            nc.vector.memset(sq[:, 0:1], 1.0)
                        sqs.append(sq)
                    state[ci] = {"tkb": tkb, "sqs": sqs}

                def phase_eq(ci):
                    st = state[ci]
                    mk = mks.tile([P, 16, 2, CF], dt.bfloat16, tag="mk")
                    for i in range(nlab):
                        nc.vector.tensor_single_scalar(
                            mk[:, i, :, :], st["tkb"][:], float(i + 1), eq
                        )
                    st["mk"] = mk

                def phase_vals(ci):
                    st = state[ci]
                    sqs, tkb = st["sqs"], st["tkb"]
                    s01 = tmp.tile([P, CF], dt.bfloat16, tag="s01")
                    nc.vector.tensor_add(s01[:], sqs[0][:], sqs[1][:])
                    s23 = tmp.tile([P, CF], dt.bfloat16, tag="s23")
                    nc.vector.tensor_add(s23[:], sqs[2][:], sqs[3][:])
                    v = tmp.tile([P, CF], dt.bfloat16, tag="v")
                    nc.vector.tensor_add(v[:], s01[:], s23[:])
                    tkm = tmp.tile([P, CF], dt.bfloat16, tag="tkm")
                    nc.vector.tensor_tensor(tkm[:], tkb[:, 0, :], tkb[:, 1, :], eq)
                    vtk = tmp.tile([P, CF], dt.bfloat16, tag="vtk")
                    nc.vector.tensor_mul(vtk[:], tkm[:], v[:])
                    st["v"], st["vtk"] = v, vtk

                def phase_cv(ci):
                    st = state[ci]
                    vt = vtp.tile([P, 2, NB, 128], dt.float8e4, tag="vt")
                    nc.vector.memset(vt[:, :, :, BC : BC + 1], 1.0)
                    for kt in range(2) if cv_on else []:
                        hs = slice(kt * HF, (kt + 1) * HF)
                        nc.scalar.copy(
                            vt[:, kt, :, 0:BC],
                            st["v"][:, hs].rearrange("p (b m) -> p b m", m=BC),
                        )
                        nc.scalar.copy(
                            vt[:, kt, :, BC + 1 : M],
                            st["vtk"][:, hs].rearrange("p (b m) -> p b m", m=BC),
                        )
                    st["vt"] = vt

                def phase_pe(ci):
                    st = state.pop(ci)
                    vt, mk = st["vt"], st["mk"]
                    mko = (
                        mk[:]
                        .bitcast(dt.float8e4)
                        .rearrange("p i s (kt c two) -> p kt i s c two", kt=2, two=2)
                    )[:, :, :, :, :, 1]
                    for blk in range(NB):
                        first = ci == 0 and blk == 0
                        last = ci == NCH - 1 and blk == NB - 1
                        lhs = vt[:, :, blk, 0:M]
                        csb = slice(blk * BC, (blk + 1) * BC)
                        for ih in range(2):
                            sl = slice(ih * 8, (ih + 1) * 8)
                            nc.tensor.matmul(
                                poT[:, ih, 0:448].rearrange("m (i c) -> m i c", c=BC),
                                lhs,
                                mko[:, :, sl, 0, csb],
                                start=first, stop=last,
                                skip_group_check=True, perf_mode=DR,
                            )
                            if pe_sets == 2:
                                nc.tensor.matmul(
                                    poK[:, ih, 0:448].rearrange("m (i c) -> m i c", c=BC),
                                    lhs,
                                    mko[:, :, sl, 1, csb],
                                    start=first, stop=last,
                                    skip_group_check=True, perf_mode=DR,
                                )

                # software pipeline: eq/cv/pe run one chunk behind load/vals
                for ci in range(NCH + 1):
                    if ci < NCH:
                        phase_load(ci)
                    if ci >= 1:
                        phase_eq(ci - 1)
                        phase_cv(ci - 1)
                    if ci < NCH:
                        phase_vals(ci)
                    if ci >= 1:
                        phase_pe(ci - 1)

                soT = tmp.tile([M, 2, 448], dt.float32, tag="soT")
                soK = tmp.tile([M, 2, 448], dt.float32, tag="soK")
                nc.scalar.copy(soT[:], poT[:, :, 0:448])
                nc.scalar.copy(soK[:], poK[:, :, 0:448])
                nc.scalar.dma_start(acc_d[b], soT[:])
                nc.scalar.dma_start(accK_d[b], soK[:])

            def _all_images():
                for b in range(IMGS):
                    _image(b)

            if reps == 1:
                _all_images()
            else:
                with tc.For_i(0, reps, 1):
                    _all_images()
    _legalize_sync(nc)
    return nc


_NC_CACHE = None


def _get_nc():
    global _NC_CACHE
    if _NC_CACHE is None:
        _NC_CACHE = _build_nc()
    return _NC_CACHE


def _decode(accT, accK):
    """accT/accK: [IMGS, M, 2, 448] -> per-image (A, Bk, Cc, cT, cK) each [16]."""
    out = []
    for b in range(IMGS):
        aT = accT[b].astype(np.float64).reshape(M, 16, BC) / MSCALE
        aK = accK[b].astype(np.float64).reshape(M, 16, BC) / MSCALE
        A = np.array([aT[c, :, c] for c in range(BC)]).sum(axis=0)
        Cc = np.array([aK[BC + 1 + c, :, c] for c in range(BC)]).sum(axis=0)
        cT = aT[BC, :, :].sum(axis=1)
        Bk = np.array([aK[c, :, c] for c in range(BC)]).sum(axis=0)
        cK = aK[BC, :, :].sum(axis=1)
        out.append((A, Bk, Cc, cT, cK))
    return out


def _finalize(per_image):
    """per_image: list of B tuples (A, Bk, Cc, cT, cK) -> float32 losses."""
    labels = np.arange(1, K_MAX + 1, dtype=np.float64)
    L_agg_tot = 0.0
    L_dis_tot = 0.0
    for A, Bk, Cc, cT, cK in per_image:
        nz = np.nonzero(cK > 0.5)[0]
        num_kernels = int(nz.max() + 1) if nz.size else 0
        valid = labels <= num_kernels

        denom = cK + 1.0
        x = A + Bk / (denom * denom) - 2.0 * Cc / denom
        pos = x > 0
        norm = np.where(pos, np.sqrt(np.where(pos, x, 1.0)), 0.0) - SIGMA_AGG
        agg_terms = np.log(norm * norm + 1.0) / (cT + 1.0)
        L_agg_tot += float(np.sum(np.where(valid, agg_terms, 0.0)))

        D = Bk / ((cK + 0.001) ** 2)
        S = D[:, None] + D[None, :]
        pair_mask = (labels[:, None] < labels[None, :]) & valid[None, :]
        pnorm = np.sqrt(np.where(pair_mask, S, 1.0))
        dnorm = SIGMA_DIS - pnorm
        dis_terms = np.log(dnorm * dnorm + 1.0)
        dis_sum = float(np.sum(np.where(pair_mask, dis_terms, 0.0)))
        if num_kernels > 1:
            nk = float(num_kernels)
            L_dis_tot += dis_sum / (nk * (nk - 1.0))
    return np.float32(L_agg_tot), np.float32(L_dis_tot)


def _run(pred_similarities, text_mask_ndi_labels, kernel_mask_ndi_labels,
         trace=False):
    sim = np.asarray(pred_similarities, dtype=np.float32).reshape(B, C, P, NFREE)
    T = np.asarray(text_mask_ndi_labels, dtype=np.int32).reshape(B, P, NFREE)
    K = np.asarray(kernel_mask_ndi_labels, dtype=np.int32).reshape(B, P, NFREE)

    in_maps = []
    for core in range(NCORES):
        s = slice(IMGS * core, IMGS * (core + 1))
        in_maps.append(
            {
                "sim": np.ascontiguousarray(sim[s]),
                "tlab": np.ascontiguousarray(T[s]),
                "klab": np.ascontiguousarray(K[s]),
            }
        )

    nc = _get_nc()
    res = run_bass_kernel_spmd(
        nc, in_maps, core_ids=list(range(NCORES)), trace=trace
    )

    per_image = []
    for core in range(NCORES):
        per_image.extend(_decode(res.results[core]["acc"], res.results[core]["accK"]))
    return _finalize(per_image), res


def kernel(pred_similarities, text_mask_ndi_labels, kernel_mask_ndi_labels):
    out, _ = _run(pred_similarities, text_mask_ndi_labels, kernel_mask_ndi_labels)
    return out
